# revision 19
# baseline (speedup 1.0000x reference)
"""Trainium2 Bass kernel for the ACTP 2-layer-LSTM + MLP rollout model.

Strategy: pure data parallel across 8 NeuronCores (batch 4096 -> 512/core),
weights replicated.  All on-chip tensors are feature-major [feat, batch] so
the time recurrence needs no transposes: matmuls are out[M,N] = W_T[K,M].T @
x[K,N] with the batch as the moving free dim (N=512).  Every concat in the
model becomes extra K-chunk matmuls accumulating into the same PSUM bank,
biases ride in "ones-row" K-chunks or the activation bias operand, and the
tiled(act,state) input of LSTM2 collapses algebraically into rows of the
h2-tail K-chunk.

Layout rules discovered on hardware:
 - matmuls with K <= ~64 anywhere in the stream permanently block the PE HAM
   clock-gate from reaching 2.4 GHz -> every K-chunk is padded to >= 96 rows
   (zero weight rows; rhs pad rows zeroed so 0*0 can't make NaN).
 - compute-engine writes at a partition offset must be 32-aligned -> the
   per-step act refresh lands at row 96 of the h2-tail chunk.
 - H=200 is split (128, 72): per-gate PSUM tiles are [128, 2, 512] (two
   banks; lanes 72..127 of the second bank hold junk that never escapes --
   the h/state writes slice [0:72]).

Only tactiles[0:10] is ever read (the model feeds back its own output after
the context window), so device I/O is tiny.  Host does all transposes.
"""

import os
import sys
import functools

sys.path.insert(0, "/opt/trn_rl_repo")

import numpy as np
import ml_dtypes

import concourse.bass as bass
from concourse import bacc
import concourse.tile as tile
from concourse import mybir
from concourse.bass_utils import run_bass_kernel_spmd

# model dims
T = 120
B = 4096
F = 48   # tactile feature size
A = 6    # action dim
H = 200  # LSTM hidden
CTX = 10
NSTEP = T - 1            # 119 scan steps
NOUT = NSTEP - (CTX - 1)  # 110 outputs
NCORES = 8
BL = B // NCORES         # 512 per-core batch
BH = BL // 2             # interleaved independent half-batch
HA = 128                 # H chunk a
HB = H - HA              # H chunk b = 72
KB = 100                 # padded K of the h*b-only chunks
KD = 109                 # K of the h2b+act+state+ones chunk
KX = 100                 # K of the x1 chunk (48 feat + ones + zeros)
G4 = 4 * H               # 800 gate rows

COMPUTE_BF16 = True

LAST_RESULT = None  # BassKernelResults of the most recent run (for test.py)

Tanh = mybir.ActivationFunctionType.Tanh
Sigmoid = mybir.ActivationFunctionType.Sigmoid


def _dt():
    return mybir.dt.bfloat16 if COMPUTE_BF16 else mybir.dt.float32


def _npdt():
    return ml_dtypes.bfloat16 if COMPUTE_BF16 else np.float32


def _build_nc():
    nc = bacc.Bacc()
    dt = _dt()
    f32 = mybir.dt.float32

    # ---- DRAM parameters (per-core shards / replicated weights) ----
    # tact: [KX, CTX, BL]: rows 0..47 tactile features, row 48 ones, rest 0
    tact = nc.declare_dram_parameter("tact", [KX, CTX, BL], dt, isOutput=False)
    acts = nc.declare_dram_parameter("acts", [A, NSTEP, BL], dt, isOutput=False)
    # statzero: row 0 = ones, rows 8..13 = state, row 14 = ones, rest zeros
    statzero = nc.declare_dram_parameter("statzero", [64, BL], dt, isOutput=False)

    wshapes = {
        "w1ha": [HA, G4], "w1hb": [KB, G4], "w1x": [KX, G4],
        "w2h2a": [HA, G4], "w2td": [KD, G4], "w2h1a": [HA, G4],
        "w2h1b": [KB, G4],
        "w3ha": [HA, H], "w3td": [KD, H], "w3x": [KX, H],
        "w4a": [HA, F], "w4b": [96, F],
    }
    wd = {k: nc.declare_dram_parameter(k, s, dt, isOutput=False)
          for k, s in wshapes.items()}
    b4 = nc.declare_dram_parameter("b4", [F, 1], f32, isOutput=False)

    out = nc.declare_dram_parameter("out", [NOUT, F, BL], f32, isOutput=True)

    from contextlib import ExitStack

    with tile.TileContext(nc) as tc, ExitStack() as ctx:
        # ---- pools ----
        wpool = ctx.enter_context(tc.tile_pool(name="wpool", bufs=1))
        stp = ctx.enter_context(tc.tile_pool(name="stp", bufs=1))
        sp = ctx.enter_context(tc.tile_pool(name="sp", bufs=2))
        op = ctx.enter_context(tc.tile_pool(name="op", bufs=4))
        pp = ctx.enter_context(tc.tile_pool(name="pp", bufs=8, space="PSUM"))

        # ---- weights to SBUF (once) ----
        W = {}
        for k, s in wshapes.items():
            W[k] = wpool.tile(s, dt, name=k.upper())
            nc.sync.dma_start(out=W[k], in_=wd[k][:, :])
        B4 = wpool.tile([F, 1], f32, name="B4")
        nc.sync.dma_start(out=B4, in_=b4[:, :])

        # ---- persistent state / combined rhs K-chunk tiles ----
        # TA1/TA2: h1a/h2a [128].  TB1: [h1b(72); zeros(28)].
        # TD: [h2b(72); zeros(24); act(6)@96; state(6); ones(1)]
        # X1: [x1(48); ones(1)@48; zeros] ; TE/TF: o3 chunks for fc2
        TACT = stp.tile([KX, CTX, BL], dt, name="TACT")
        ACTS = stp.tile([A, NSTEP, BL], dt, name="ACTS")
        nc.sync.dma_start(out=TACT, in_=tact[:, :, :])
        nc.sync.dma_start(out=ACTS, in_=acts[:, :, :])
        halves = []
        for hx in range(2):
            cs = slice(hx * BH, (hx + 1) * BH)
            hh = {}
            hh["cs"] = cs
            hh["TA1"] = stp.tile([HA, BH], dt, name=f"TA1_{hx}")
            hh["TB1"] = stp.tile([KB, BH], dt, name=f"TB1_{hx}")
            hh["TA2"] = stp.tile([HA, BH], dt, name=f"TA2_{hx}")
            hh["TD"] = stp.tile([KD, BH], dt, name=f"TD_{hx}")
            hh["c1"] = stp.tile([HA, 2, BH], f32, name=f"c1_{hx}")
            hh["c2"] = stp.tile([HA, 2, BH], f32, name=f"c2_{hx}")
            hh["x1"] = stp.tile([KX, BH], dt, name=f"x1_{hx}")
            hh["TE"] = stp.tile([HA, BH], dt, name=f"TE_{hx}")
            hh["TF"] = stp.tile([96, BH], dt, name=f"TF_{hx}")
            nc.sync.dma_start(out=hh["x1"][F:KX, :], in_=statzero[0:KX - F, cs])
            nc.sync.dma_start(out=hh["TD"][96 + A:KD, :],
                              in_=statzero[8:8 + A + 1, cs])
            nc.vector.memset(hh["TA1"], 0.0)
            nc.vector.memset(hh["TB1"], 0.0)
            nc.vector.memset(hh["TA2"], 0.0)
            nc.vector.memset(hh["TD"][0:96, :], 0.0)
            nc.vector.memset(hh["TF"], 0.0)
            nc.vector.memset(hh["c1"], 0.0)
            nc.vector.memset(hh["c2"], 0.0)
            halves.append(hh)

        # gate column layout (permuted rows [i f o g], chunks a=128/b=72)
        GBASE = {"i": 0, "f": 200, "o": 400, "g": 600}

        def lstm_gates(kchunks, tag):
            """kchunks: list of (weight_key, rhs) accumulated in order.
            Per gate one [128, 2, BH] PSUM tile (one bank): slot0 = a-chunk
            (M=128), slot1 = b-chunk (M=72, lanes 72..127 junk)."""
            P = {}
            for gate in ("g", "i", "f", "o"):
                gp = pp.tile([HA, 2, BH], f32, name=f"P{gate}_{tag}", tag="g")
                for m, (mo, mn) in enumerate(((0, HA), (HA, H))):
                    ps = gp[0:mn - mo, m, :]
                    cols = slice(GBASE[gate] + mo, GBASE[gate] + mn)
                    last = len(kchunks) - 1
                    for j, (wk, rhs) in enumerate(kchunks):
                        nc.tensor.matmul(ps, W[wk][:, cols], rhs,
                                         start=(j == 0), stop=(j == last))
                P[gate] = gp
            return P

        def lstm_cell(P, c, ha, hb, tag):
            """update c (f32 [128,2,BH]) and h (ha [128,BH], hb [72,BH])"""
            gt = sp.tile([HA, 2, BH], dt, name=f"gt{tag}", tag="gt")
            sgi = sp.tile([HA, 2, BH], dt, name=f"sgi{tag}", tag="sgi")
            sgf = sp.tile([HA, 2, BH], dt, name=f"sgf{tag}", tag="sgf")
            sgo = sp.tile([HA, 2, BH], dt, name=f"sgo{tag}", tag="sgo")
            nc.scalar.activation(gt, P["g"], Tanh)
            nc.scalar.activation(sgi, P["i"], Sigmoid)
            nc.scalar.activation(sgf, P["f"], Sigmoid)
            nc.scalar.activation(sgo, P["o"], Sigmoid)
            ig = sp.tile([HA, 2, BH], dt, name=f"ig{tag}", tag="ig")
            fm = sp.tile([HA, 2, BH], f32, name=f"fm{tag}", tag="fm")
            nc.vector.tensor_mul(ig, sgi, gt)
            nc.vector.tensor_mul(fm, sgf, c)
            nc.vector.tensor_add(c, fm, ig)
            tch = sp.tile([HA, 2, BH], dt, name=f"tch{tag}", tag="tch")
            nc.scalar.activation(tch, c, Tanh)
            nc.vector.tensor_mul(ha, sgo[:, 0, :], tch[:, 0, :])
            nc.vector.tensor_mul(hb, sgo[0:HB, 1, :], tch[0:HB, 1, :])

        def emit_lstm1(hh, t, hx):
            x1_rhs = TACT[:, t, hh["cs"]] if t < CTX else hh["x1"]
            nc.vector.tensor_copy(hh["TD"][96:96 + A, :],
                                  ACTS[:, t, hh["cs"]])
            P1 = lstm_gates([("w1ha", hh["TA1"]), ("w1hb", hh["TB1"]),
                             ("w1x", x1_rhs)], f"1_{t}_{hx}")
            lstm_cell(P1, hh["c1"], hh["TA1"], hh["TB1"][0:HB, :],
                      f"1_{t}_{hx}")

        def emit_lstm2(hh, t, hx):
            P2 = lstm_gates([("w2h2a", hh["TA2"]), ("w2td", hh["TD"]),
                             ("w2h1a", hh["TA1"]), ("w2h1b", hh["TB1"])],
                            f"2_{t}_{hx}")
            lstm_cell(P2, hh["c2"], hh["TA2"], hh["TD"][0:HB, :],
                      f"2_{t}_{hx}")

        def emit_fc(hh, t, hx):
            x1_rhs = TACT[:, t, hh["cs"]] if t < CTX else hh["x1"]
            fcp = pp.tile([HA, 2, BH], f32, name=f"fcp_{t}_{hx}", tag="g")
            for m, (mo, mn) in enumerate(((0, HA), (HA, H))):
                ps = fcp[0:mn - mo, m, :]
                cols = slice(mo, mn)
                nc.tensor.matmul(ps, W["w3x"][:, cols], x1_rhs,
                                 start=True, stop=False)
                nc.tensor.matmul(ps, W["w3ha"][:, cols], hh["TA2"],
                                 start=False, stop=False)
                nc.tensor.matmul(ps, W["w3td"][:, cols], hh["TD"],
                                 start=False, stop=True)
            nc.scalar.activation(hh["TE"], fcp[:, 0, :], Tanh)
            nc.scalar.activation(hh["TF"][0:HB, :], fcp[0:HB, 1, :], Tanh)
            f2p = pp.tile([F, BH], f32, name=f"f2p_{t}_{hx}", tag="g")
            nc.tensor.matmul(f2p, W["w4a"], hh["TE"], start=True, stop=False)
            nc.tensor.matmul(f2p, W["w4b"], hh["TF"], start=False, stop=True)
            stg = op.tile([F, BH], f32, name=f"stg_{t}_{hx}", tag="stg")
            if t < NSTEP - 1:
                nc.scalar.activation(hh["x1"][0:F, :], f2p, Tanh, bias=B4)
                nc.vector.tensor_copy(stg, hh["x1"][0:F, :])
            else:
                nc.scalar.activation(stg, f2p, Tanh, bias=B4)
            nc.gpsimd.dma_start(out=out[t - (CTX - 1), :, hh["cs"]], in_=stg)

        X, Y = halves
        # zipper the two independent half-batch recurrences: each half's fc
        # block is emitted inside the other half's LSTM1 window so the PE
        # FIFO always holds runnable matmuls during dependency chains
        for t in range(NSTEP):
            emit_lstm1(X, t, 0)
            if t - 1 >= CTX - 1:
                emit_fc(Y, t - 1, 1)
            emit_lstm1(Y, t, 1)
            emit_lstm2(X, t, 0)
            emit_lstm2(Y, t, 1)
            if t >= CTX - 1:
                emit_fc(X, t, 0)
        emit_fc(Y, NSTEP - 1, 1)

    nc.finalize()
    return nc


@functools.lru_cache(maxsize=1)
def _get_nc():
    return _build_nc()


def _prep_weights(W_ih1, W_hh1, b_ih1, b_hh1, W_ih2, W_hh2, b_ih2, b_hh2,
                  fc1_w, fc1_b, fc2_w, fc2_b):
    # gate rows reordered [i, f, o, g]
    perm = np.concatenate([np.arange(0, 200), np.arange(200, 400),
                           np.arange(600, 800), np.arange(400, 600)])
    W1p = np.asarray(W_ih1)[perm]          # [800, 48]
    W1hp = np.asarray(W_hh1)[perm]         # [800, 200]
    b1p = (np.asarray(b_ih1) + np.asarray(b_hh1))[perm]
    W2p = np.asarray(W_ih2)[perm]          # [800, 248]
    W2hp = np.asarray(W_hh2)[perm]         # [800, 200]
    b2p = (np.asarray(b_ih2) + np.asarray(b_hh2))[perm]
    Wt = W2p[:, 200:248]
    W2eff = Wt[:, 0:12] + Wt[:, 12:24] + Wt[:, 24:36] + Wt[:, 36:48]  # [800,12]
    fc1_w = np.asarray(fc1_w); fc1_b = np.asarray(fc1_b)
    fc2_w = np.asarray(fc2_w); fc2_b = np.asarray(fc2_b)
    npdt = _npdt()

    def c(x):
        return np.ascontiguousarray(x).astype(npdt)

    def pad_to(x, k):
        z = np.zeros((k, x.shape[1]), np.float32)
        z[:x.shape[0]] = x
        return z

    def td_weights(w_hb_T, w_as_T, bias):
        z = np.zeros((KD, w_hb_T.shape[1]), np.float32)
        z[0:HB] = w_hb_T
        z[96:108] = w_as_T
        z[108] = bias
        return z

    return {
        "w1ha": c(W1hp[:, 0:HA].T),
        "w1hb": c(pad_to(W1hp[:, HA:H].T, KB)),
        "w1x": c(pad_to(np.concatenate([W1p.T, b1p[None, :]], 0), KX)),
        "w2h2a": c(W2hp[:, 0:HA].T),
        "w2td": c(td_weights(W2hp[:, HA:H].T, W2eff.T, b2p)),
        "w2h1a": c(W2p[:, 0:HA].T),
        "w2h1b": c(pad_to(W2p[:, HA:H].T, KB)),
        "w3ha": c(fc1_w[:, 0:HA].T),
        "w3td": c(pad_to(fc1_w[:, HA:H].T, KD)),
        "w3x": c(pad_to(np.concatenate([fc1_w[:, 200:248].T,
                                        fc1_b[None, :]], 0), KX)),
        "w4a": c(fc2_w[:, 0:HA].T),
        "w4b": c(pad_to(fc2_w[:, HA:H].T, 96)),
        "b4": np.ascontiguousarray(fc2_b[:, None]).astype(np.float32),
    }


def kernel(tactiles, actions, W_ih1, W_hh1, b_ih1, b_hh1,
           W_ih2, W_hh2, b_ih2, b_hh2, fc1_w, fc1_b, fc2_w, fc2_b):
    global LAST_RESULT
    npdt = _npdt()
    tactiles = np.asarray(tactiles)
    actions = np.asarray(actions)

    wmap = _prep_weights(W_ih1, W_hh1, b_ih1, b_hh1, W_ih2, W_hh2, b_ih2, b_hh2,
                         fc1_w, fc1_b, fc2_w, fc2_b)

    in_maps = []
    for i in range(NCORES):
        s = slice(i * BL, (i + 1) * BL)
        tt = np.zeros((KX, CTX, BL), np.float32)
        tt[0:F] = tactiles[0:CTX, s, :].transpose(2, 0, 1)
        tt[F] = 1.0
        acts_T = np.ascontiguousarray(
            actions[1:T, s, :].transpose(2, 0, 1)).astype(npdt)   # [6,119,BL]
        sz = np.zeros((64, BL), np.float32)
        sz[0] = 1.0                      # x1 ones row
        sz[8:8 + A] = actions[0, s, :].T  # state rows
        sz[8 + A] = 1.0                  # TD ones row
        m = {"tact": tt.astype(npdt), "acts": acts_T,
             "statzero": sz.astype(npdt)}
        m.update(wmap)
        in_maps.append(m)

    nc = _get_nc()
    res = run_bass_kernel_spmd(nc, in_maps, core_ids=list(range(NCORES)))
    LAST_RESULT = res

    outs = [np.asarray(r["out"], dtype=np.float32) for r in res.results]
    full = np.concatenate([o.transpose(0, 2, 1) for o in outs], axis=1)
    return np.ascontiguousarray(full)


# revision 24
# speedup vs baseline: 1.0357x; 1.0357x over previous
"""Trainium2 Bass kernel for the ACTP 2-layer-LSTM + MLP rollout model.

Strategy: pure data parallel across 8 NeuronCores (batch 4096 -> 512/core),
weights replicated.  All on-chip tensors are feature-major [feat, batch] so
the time recurrence needs no transposes: matmuls are out[M,N] = W_T[K,M].T @
x[K,N] with the batch as the moving free dim (N=512).  Every concat in the
model becomes extra K-chunk matmuls accumulating into the same PSUM bank,
biases ride in "ones-row" K-chunks or the activation bias operand, and the
tiled(act,state) input of LSTM2 collapses algebraically into rows of the
h2-tail K-chunk.

Layout rules discovered on hardware:
 - matmuls with K <= ~64 anywhere in the stream permanently block the PE HAM
   clock-gate from reaching 2.4 GHz -> every K-chunk is padded to >= 96 rows
   (zero weight rows; rhs pad rows zeroed so 0*0 can't make NaN).
 - compute-engine writes at a partition offset must be 32-aligned -> the
   per-step act refresh lands at row 96 of the h2-tail chunk.
 - H=200 is split (128, 72): per-gate PSUM tiles are [128, 2, 512] (two
   banks; lanes 72..127 of the second bank hold junk that never escapes --
   the h/state writes slice [0:72]).

Only tactiles[0:10] is ever read (the model feeds back its own output after
the context window), so device I/O is tiny.  Host does all transposes.
"""

import os
import sys
import functools

sys.path.insert(0, "/opt/trn_rl_repo")

import numpy as np
import ml_dtypes

import concourse.bass as bass
from concourse import bacc
import concourse.tile as tile
from concourse import mybir
from concourse.bass_utils import run_bass_kernel_spmd

# model dims
T = 120
B = 4096
F = 48   # tactile feature size
A = 6    # action dim
H = 200  # LSTM hidden
CTX = 10
NSTEP = T - 1            # 119 scan steps
NOUT = NSTEP - (CTX - 1)  # 110 outputs
NCORES = 8
BL = B // NCORES         # 512 per-core batch
BH = BL // 2             # interleaved independent half-batch
HA = 128                 # H chunk a
HB = H - HA              # H chunk b = 72
KB = 100                 # padded K of the h*b-only chunks
KD = 109                 # K of the h2b+act+state+ones chunk
KX = 100                 # K of the x1 chunk (48 feat + ones + zeros)
G4 = 4 * H               # 800 gate rows

COMPUTE_BF16 = True

LAST_RESULT = None  # BassKernelResults of the most recent run (for test.py)

Tanh = mybir.ActivationFunctionType.Tanh
Sigmoid = mybir.ActivationFunctionType.Sigmoid


def _dt():
    return mybir.dt.bfloat16 if COMPUTE_BF16 else mybir.dt.float32


def _npdt():
    return ml_dtypes.bfloat16 if COMPUTE_BF16 else np.float32


def _build_nc():
    nc = bacc.Bacc()
    dt = _dt()
    f32 = mybir.dt.float32

    # ---- DRAM parameters (per-core shards / replicated weights) ----
    # tact: [KX, CTX, BL]: rows 0..47 tactile features, row 48 ones, rest 0
    tact = nc.declare_dram_parameter("tact", [KX, CTX, BL], dt, isOutput=False)
    acts = nc.declare_dram_parameter("acts", [A, NSTEP, BL], dt, isOutput=False)
    # statzero: row 0 = ones, rows 8..13 = state, row 14 = ones, rest zeros
    statzero = nc.declare_dram_parameter("statzero", [64, BL], dt, isOutput=False)

    wshapes = {
        "w1ha": [HA, G4], "w1hb": [KB, G4], "w1x": [KX, G4],
        "w2h2a": [HA, G4], "w2td": [KD, G4], "w2h1a": [HA, G4],
        "w2h1b": [KB, G4],
        "w3ha": [HA, H], "w3td": [KD, H], "w3x": [KX, H],
        "w4a": [HA, F], "w4b": [96, F],
    }
    wd = {k: nc.declare_dram_parameter(k, s, dt, isOutput=False)
          for k, s in wshapes.items()}
    b4 = nc.declare_dram_parameter("b4", [F, 1], f32, isOutput=False)

    out = nc.declare_dram_parameter("out", [NOUT, F, BL], f32, isOutput=True)

    from contextlib import ExitStack

    with tile.TileContext(nc) as tc, ExitStack() as ctx:
        # ---- pools ----
        wpool = ctx.enter_context(tc.tile_pool(name="wpool", bufs=1))
        stp = ctx.enter_context(tc.tile_pool(name="stp", bufs=1))
        sp = ctx.enter_context(tc.tile_pool(name="sp", bufs=2))
        op = ctx.enter_context(tc.tile_pool(name="op", bufs=4))
        pp = ctx.enter_context(tc.tile_pool(name="pp", bufs=8, space="PSUM"))

        # ---- weights to SBUF (once) ----
        W = {}
        for k, s in wshapes.items():
            W[k] = wpool.tile(s, dt, name=k.upper())
            nc.sync.dma_start(out=W[k], in_=wd[k][:, :])
        B4 = wpool.tile([F, 1], f32, name="B4")
        nc.sync.dma_start(out=B4, in_=b4[:, :])

        # ---- persistent state / combined rhs K-chunk tiles ----
        # TA1/TA2: h1a/h2a [128].  TB1: [h1b(72); zeros(28)].
        # TD: [h2b(72); zeros(24); act(6)@96; state(6); ones(1)]
        # X1: [x1(48); ones(1)@48; zeros] ; TE/TF: o3 chunks for fc2
        TACT = stp.tile([KX, CTX, BL], dt, name="TACT")
        ACTS = stp.tile([A, NSTEP, BL], dt, name="ACTS")
        nc.sync.dma_start(out=TACT, in_=tact[:, :, :])
        nc.sync.dma_start(out=ACTS, in_=acts[:, :, :])
        halves = []
        for hx in range(2):
            cs = slice(hx * BH, (hx + 1) * BH)
            hh = {}
            hh["cs"] = cs
            hh["TA1"] = stp.tile([HA, BH], dt, name=f"TA1_{hx}")
            hh["TB1"] = stp.tile([KB, BH], dt, name=f"TB1_{hx}")
            hh["TA2"] = stp.tile([HA, BH], dt, name=f"TA2_{hx}")
            hh["TD"] = stp.tile([KD, BH], dt, name=f"TD_{hx}")
            hh["c1"] = stp.tile([HA, 2, BH], dt, name=f"c1_{hx}")
            hh["c2"] = stp.tile([HA, 2, BH], dt, name=f"c2_{hx}")
            hh["x1"] = stp.tile([KX, BH], dt, name=f"x1_{hx}")
            hh["TEF"] = stp.tile([HA, 2, BH], dt, name=f"TEF_{hx}")
            nc.sync.dma_start(out=hh["x1"][F:KX, :], in_=statzero[0:KX - F, cs])
            nc.sync.dma_start(out=hh["TD"][96 + A:KD, :],
                              in_=statzero[8:8 + A + 1, cs])
            nc.vector.memset(hh["TA1"], 0.0)
            nc.vector.memset(hh["TB1"], 0.0)
            nc.vector.memset(hh["TA2"], 0.0)
            nc.vector.memset(hh["TD"][0:96, :], 0.0)
            nc.vector.memset(hh["TEF"], 0.0)
            nc.vector.memset(hh["c1"], 0.0)
            nc.vector.memset(hh["c2"], 0.0)
            halves.append(hh)

        # zero all PSUM banks once: lanes the matmuls never write then
        # read as 0.0 forever (removes junk-lane hazards)
        for zb in range(8):
            pz = pp.tile([HA, 2, BH], f32, name=f"pz_{zb}", tag="g")
            nc.vector.memset(pz, 0.0)

        # gate column layout (permuted rows [i f o g], chunks a=128/b=72)
        GBASE = {"i": 0, "f": 200, "o": 400, "g": 600}

        def lstm_gates(kchunks, tag):
            """kchunks: list of (weight_key, rhs) accumulated in order.
            Per gate one [128, 2, BH] PSUM tile (one bank): slot0 = a-chunk
            (M=128), slot1 = b-chunk (M=72, lanes 72..127 junk)."""
            P = {}
            for gate in ("g", "i", "f", "o"):
                gp = pp.tile([HA, 2, BH], f32, name=f"P{gate}_{tag}", tag="g")
                for m, (mo, mn) in enumerate(((0, HA), (HA, H))):
                    ps = gp[0:mn - mo, m, :]
                    cols = slice(GBASE[gate] + mo, GBASE[gate] + mn)
                    last = len(kchunks) - 1
                    for j, (wk, rhs) in enumerate(kchunks):
                        nc.tensor.matmul(ps, W[wk][:, cols], rhs,
                                         start=(j == 0), stop=(j == last))
                P[gate] = gp
            return P

        def lstm_cell(P, c, ha, hb, tag):
            """update c (f32 [128,2,BH]) and h (ha [128,BH], hb [72,BH])"""
            gt = sp.tile([HA, 2, BH], dt, name=f"gt{tag}", tag="gt")
            sgi = sp.tile([HA, 2, BH], dt, name=f"sgi{tag}", tag="sgi")
            sgf = sp.tile([HA, 2, BH], dt, name=f"sgf{tag}", tag="sgf")
            sgo = sp.tile([HA, 2, BH], dt, name=f"sgo{tag}", tag="sgo")
            nc.scalar.activation(gt, P["g"], Tanh)
            nc.scalar.activation(sgi, P["i"], Sigmoid)
            nc.scalar.activation(sgf, P["f"], Sigmoid)
            nc.scalar.activation(sgo, P["o"], Sigmoid)
            ig = sp.tile([HA, 2, BH], dt, name=f"ig{tag}", tag="ig")
            fm = sp.tile([HA, 2, BH], dt, name=f"fm{tag}", tag="fm")
            nc.vector.tensor_mul(ig, sgi, gt)
            nc.vector.tensor_mul(fm, sgf, c)
            nc.vector.tensor_add(c, fm, ig)
            tch = sp.tile([HA, 2, BH], dt, name=f"tch{tag}", tag="tch")
            nc.scalar.activation(tch, c, Tanh)
            nc.vector.tensor_mul(ha, sgo[:, 0, :], tch[:, 0, :])
            nc.vector.tensor_mul(hb, sgo[0:HB, 1, :], tch[0:HB, 1, :])

        def emit_lstm1(hh, t, hx):
            x1_rhs = TACT[:, t, hh["cs"]] if t < CTX else hh["x1"]
            nc.vector.tensor_copy(hh["TD"][96:96 + A, :],
                                  ACTS[:, t, hh["cs"]])
            P1 = lstm_gates([("w1ha", hh["TA1"]), ("w1hb", hh["TB1"]),
                             ("w1x", x1_rhs)], f"1_{t}_{hx}")
            lstm_cell(P1, hh["c1"], hh["TA1"], hh["TB1"][0:HB, :],
                      f"1_{t}_{hx}")

        def emit_lstm2(hh, t, hx):
            P2 = lstm_gates([("w2h2a", hh["TA2"]), ("w2td", hh["TD"]),
                             ("w2h1a", hh["TA1"]), ("w2h1b", hh["TB1"])],
                            f"2_{t}_{hx}")
            lstm_cell(P2, hh["c2"], hh["TA2"], hh["TD"][0:HB, :],
                      f"2_{t}_{hx}")

        def emit_fc(hh, t, hx):
            x1_rhs = TACT[:, t, hh["cs"]] if t < CTX else hh["x1"]
            fcp = pp.tile([HA, 2, BH], f32, name=f"fcp_{t}_{hx}", tag="g")
            for m, (mo, mn) in enumerate(((0, HA), (HA, H))):
                ps = fcp[0:mn - mo, m, :]
                cols = slice(mo, mn)
                nc.tensor.matmul(ps, W["w3x"][:, cols], x1_rhs,
                                 start=True, stop=False)
                nc.tensor.matmul(ps, W["w3ha"][:, cols], hh["TA2"],
                                 start=False, stop=False)
                nc.tensor.matmul(ps, W["w3td"][:, cols], hh["TD"],
                                 start=False, stop=True)
            nc.scalar.activation(hh["TEF"], fcp, Tanh)
            f2p = pp.tile([F, BH], f32, name=f"f2p_{t}_{hx}", tag="g")
            nc.tensor.matmul(f2p, W["w4a"], hh["TEF"][:, 0, :],
                             start=True, stop=False)
            nc.tensor.matmul(f2p, W["w4b"], hh["TEF"][0:96, 1, :],
                             start=False, stop=True)
            stg = op.tile([F, BH], f32, name=f"stg_{t}_{hx}", tag="stg")
            if t < NSTEP - 1:
                nc.scalar.activation(hh["x1"][0:F, :], f2p, Tanh, bias=B4)
                nc.vector.tensor_copy(stg, hh["x1"][0:F, :])
            else:
                nc.scalar.activation(stg, f2p, Tanh, bias=B4)
            nc.gpsimd.dma_start(out=out[t - (CTX - 1), :, hh["cs"]], in_=stg)

        X, Y = halves
        # zipper the two independent half-batch recurrences: each half's fc
        # block is emitted inside the other half's LSTM1 window so the PE
        # FIFO always holds runnable matmuls during dependency chains
        for t in range(NSTEP):
            emit_lstm1(X, t, 0)
            if t - 1 >= CTX - 1:
                emit_fc(Y, t - 1, 1)
            emit_lstm1(Y, t, 1)
            emit_lstm2(X, t, 0)
            emit_lstm2(Y, t, 1)
            if t >= CTX - 1:
                emit_fc(X, t, 0)
        emit_fc(Y, NSTEP - 1, 1)

    nc.finalize()
    return nc


@functools.lru_cache(maxsize=1)
def _get_nc():
    return _build_nc()


def _prep_weights(W_ih1, W_hh1, b_ih1, b_hh1, W_ih2, W_hh2, b_ih2, b_hh2,
                  fc1_w, fc1_b, fc2_w, fc2_b):
    # gate rows reordered [i, f, o, g]
    perm = np.concatenate([np.arange(0, 200), np.arange(200, 400),
                           np.arange(600, 800), np.arange(400, 600)])
    W1p = np.asarray(W_ih1)[perm]          # [800, 48]
    W1hp = np.asarray(W_hh1)[perm]         # [800, 200]
    b1p = (np.asarray(b_ih1) + np.asarray(b_hh1))[perm]
    W2p = np.asarray(W_ih2)[perm]          # [800, 248]
    W2hp = np.asarray(W_hh2)[perm]         # [800, 200]
    b2p = (np.asarray(b_ih2) + np.asarray(b_hh2))[perm]
    Wt = W2p[:, 200:248]
    W2eff = Wt[:, 0:12] + Wt[:, 12:24] + Wt[:, 24:36] + Wt[:, 36:48]  # [800,12]
    fc1_w = np.asarray(fc1_w); fc1_b = np.asarray(fc1_b)
    fc2_w = np.asarray(fc2_w); fc2_b = np.asarray(fc2_b)
    npdt = _npdt()

    def c(x):
        return np.ascontiguousarray(x).astype(npdt)

    def pad_to(x, k):
        z = np.zeros((k, x.shape[1]), np.float32)
        z[:x.shape[0]] = x
        return z

    def td_weights(w_hb_T, w_as_T, bias):
        z = np.zeros((KD, w_hb_T.shape[1]), np.float32)
        z[0:HB] = w_hb_T
        z[96:108] = w_as_T
        z[108] = bias
        return z

    return {
        "w1ha": c(W1hp[:, 0:HA].T),
        "w1hb": c(pad_to(W1hp[:, HA:H].T, KB)),
        "w1x": c(pad_to(np.concatenate([W1p.T, b1p[None, :]], 0), KX)),
        "w2h2a": c(W2hp[:, 0:HA].T),
        "w2td": c(td_weights(W2hp[:, HA:H].T, W2eff.T, b2p)),
        "w2h1a": c(W2p[:, 0:HA].T),
        "w2h1b": c(pad_to(W2p[:, HA:H].T, KB)),
        "w3ha": c(fc1_w[:, 0:HA].T),
        "w3td": c(pad_to(fc1_w[:, HA:H].T, KD)),
        "w3x": c(pad_to(np.concatenate([fc1_w[:, 200:248].T,
                                        fc1_b[None, :]], 0), KX)),
        "w4a": c(fc2_w[:, 0:HA].T),
        "w4b": c(pad_to(fc2_w[:, HA:H].T, 96)),
        "b4": np.ascontiguousarray(fc2_b[:, None]).astype(np.float32),
    }


def kernel(**inputs):
    """Full-input entry point.  The Tile scheduler makes hash-order-
    dependent choices worth ~20% kernel time; re-exec under a pinned
    PYTHONHASHSEED unless already pinned (falls back to in-process)."""
    if os.environ.get("PYTHONHASHSEED") != "0":
        try:
            return _kernel_subprocess(inputs)
        except Exception as e:
            print("kernel: subprocess path failed (%s); running in-process" % e)
    return _kernel_impl(**inputs)


def _kernel_subprocess(inputs):
    import subprocess
    import tempfile
    d = tempfile.mkdtemp(prefix="actp_kernel_")
    inp = os.path.join(d, "in.npz")
    outp = os.path.join(d, "out.npy")
    np.savez(inp, **{k: np.asarray(v) for k, v in inputs.items()})
    env = dict(os.environ)
    env["PYTHONHASHSEED"] = "0"
    code = (
        "import numpy as np, importlib.util\n"
        "spec = importlib.util.spec_from_file_location('actp_kmod', %r)\n"
        "m = importlib.util.module_from_spec(spec)\n"
        "spec.loader.exec_module(m)\n"
        "d = np.load(%r)\n"
        "out = m.kernel(**{k: d[k] for k in d.files})\n"
        "np.save(%r, out)\n" % (os.path.abspath(__file__), inp, outp)
    )
    subprocess.run([sys.executable, "-c", code], env=env, check=True,
                   timeout=3000)
    return np.load(outp)


def _kernel_impl(tactiles, actions, W_ih1, W_hh1, b_ih1, b_hh1,
                 W_ih2, W_hh2, b_ih2, b_hh2, fc1_w, fc1_b, fc2_w, fc2_b):
    global LAST_RESULT
    npdt = _npdt()
    tactiles = np.asarray(tactiles)
    actions = np.asarray(actions)

    wmap = _prep_weights(W_ih1, W_hh1, b_ih1, b_hh1, W_ih2, W_hh2, b_ih2, b_hh2,
                         fc1_w, fc1_b, fc2_w, fc2_b)

    in_maps = []
    for i in range(NCORES):
        s = slice(i * BL, (i + 1) * BL)
        tt = np.zeros((KX, CTX, BL), np.float32)
        tt[0:F] = tactiles[0:CTX, s, :].transpose(2, 0, 1)
        tt[F] = 1.0
        acts_T = np.ascontiguousarray(
            actions[1:T, s, :].transpose(2, 0, 1)).astype(npdt)   # [6,119,BL]
        sz = np.zeros((64, BL), np.float32)
        sz[0] = 1.0                      # x1 ones row
        sz[8:8 + A] = actions[0, s, :].T  # state rows
        sz[8 + A] = 1.0                  # TD ones row
        m = {"tact": tt.astype(npdt), "acts": acts_T,
             "statzero": sz.astype(npdt)}
        m.update(wmap)
        in_maps.append(m)

    nc = _get_nc()
    res = run_bass_kernel_spmd(nc, in_maps, core_ids=list(range(NCORES)))
    LAST_RESULT = res

    outs = [np.asarray(r["out"], dtype=np.float32) for r in res.results]
    full = np.concatenate([o.transpose(0, 2, 1) for o in outs], axis=1)
    return np.ascontiguousarray(full)


# revision 25
# speedup vs baseline: 1.2396x; 1.1969x over previous
"""Trainium2 Bass kernel for the ACTP 2-layer-LSTM + MLP rollout model.

Strategy: pure data parallel across 8 NeuronCores (batch 4096 -> 512/core),
weights replicated.  All on-chip tensors are feature-major [feat, batch] so
the time recurrence needs no transposes: matmuls are out[M,N] = W_T[K,M].T @
x[K,N] with the batch as the moving free dim (N=512).  Every concat in the
model becomes extra K-chunk matmuls accumulating into the same PSUM bank,
biases ride in "ones-row" K-chunks or the activation bias operand, and the
tiled(act,state) input of LSTM2 collapses algebraically into rows of the
h2-tail K-chunk.

Layout rules discovered on hardware:
 - matmuls with K <= ~64 anywhere in the stream permanently block the PE HAM
   clock-gate from reaching 2.4 GHz -> every K-chunk is padded to >= 96 rows
   (zero weight rows; rhs pad rows zeroed so 0*0 can't make NaN).
 - compute-engine writes at a partition offset must be 32-aligned -> the
   per-step act refresh lands at row 96 of the h2-tail chunk.
 - H=200 is split (128, 72): per-gate PSUM tiles are [128, 2, 512] (two
   banks; lanes 72..127 of the second bank hold junk that never escapes --
   the h/state writes slice [0:72]).

Only tactiles[0:10] is ever read (the model feeds back its own output after
the context window), so device I/O is tiny.  Host does all transposes.
"""

import os
import sys
import functools

sys.path.insert(0, "/opt/trn_rl_repo")

import numpy as np
import ml_dtypes

import concourse.bass as bass
from concourse import bacc
import concourse.tile as tile
from concourse import mybir
from concourse.bass_utils import run_bass_kernel_spmd

# model dims
T = 120
B = 4096
F = 48   # tactile feature size
A = 6    # action dim
H = 200  # LSTM hidden
CTX = 10
NSTEP = T - 1            # 119 scan steps
NOUT = NSTEP - (CTX - 1)  # 110 outputs
NCORES = 8
BL = B // NCORES         # 512 per-core batch
BH = BL // 2             # interleaved independent half-batch
HA = 128                 # H chunk a
HB = H - HA              # H chunk b = 72
KB = 100                 # padded K of the h*b-only chunks
KD = 109                 # K of the h2b+act+state+ones chunk
KX = 100                 # K of the x1 chunk (48 feat + ones + zeros)
G4 = 4 * H               # 800 gate rows

COMPUTE_BF16 = True

LAST_RESULT = None  # BassKernelResults of the most recent run (for test.py)

Tanh = mybir.ActivationFunctionType.Tanh
Sigmoid = mybir.ActivationFunctionType.Sigmoid


def _dt():
    return mybir.dt.bfloat16 if COMPUTE_BF16 else mybir.dt.float32


def _npdt():
    return ml_dtypes.bfloat16 if COMPUTE_BF16 else np.float32


def _build_nc():
    nc = bacc.Bacc()
    dt = _dt()
    f32 = mybir.dt.float32

    # ---- DRAM parameters (per-core shards / replicated weights) ----
    # tact: [KX, CTX, BL]: rows 0..47 tactile features, row 48 ones, rest 0
    tact = nc.declare_dram_parameter("tact", [KX, CTX, BL], dt, isOutput=False)
    acts = nc.declare_dram_parameter("acts", [A, NSTEP, BL], dt, isOutput=False)
    # statzero: row 0 = ones, rows 8..13 = state, row 14 = ones, rest zeros
    statzero = nc.declare_dram_parameter("statzero", [64, BL], dt, isOutput=False)

    wshapes = {
        "w1ha": [HA, G4], "w1hb": [KB, G4], "w1x": [KX, G4],
        "w2h2a": [HA, G4], "w2td": [KD, G4], "w2h1a": [HA, G4],
        "w2h1b": [KB, G4],
        "w3ha": [HA, H], "w3td": [KD, H], "w3x": [KX, H],
        "w4a": [HA, F], "w4b": [96, F],
    }
    wd = {k: nc.declare_dram_parameter(k, s, dt, isOutput=False)
          for k, s in wshapes.items()}
    b4 = nc.declare_dram_parameter("b4", [F, 1], f32, isOutput=False)

    out = nc.declare_dram_parameter("out", [NOUT, F, BL], f32, isOutput=True)

    from contextlib import ExitStack

    with tile.TileContext(nc) as tc, ExitStack() as ctx:
        # ---- pools ----
        wpool = ctx.enter_context(tc.tile_pool(name="wpool", bufs=1))
        stp = ctx.enter_context(tc.tile_pool(name="stp", bufs=1))
        sp = ctx.enter_context(tc.tile_pool(name="sp", bufs=2))
        op = ctx.enter_context(tc.tile_pool(name="op", bufs=4))
        pp = ctx.enter_context(tc.tile_pool(name="pp", bufs=8, space="PSUM"))

        # ---- weights to SBUF (once) ----
        W = {}
        for k, s in wshapes.items():
            W[k] = wpool.tile(s, dt, name=k.upper())
            nc.sync.dma_start(out=W[k], in_=wd[k][:, :])
        B4 = wpool.tile([F, 1], f32, name="B4")
        nc.sync.dma_start(out=B4, in_=b4[:, :])

        # ---- persistent state / combined rhs K-chunk tiles ----
        # TA1/TA2: h1a/h2a [128].  TB1: [h1b(72); zeros(28)].
        # TD: [h2b(72); zeros(24); act(6)@96; state(6); ones(1)]
        # X1: [x1(48); ones(1)@48; zeros] ; TE/TF: o3 chunks for fc2
        TACT = stp.tile([KX, CTX, BL], dt, name="TACT")
        ACTS = stp.tile([A, NSTEP, BL], dt, name="ACTS")
        nc.sync.dma_start(out=TACT, in_=tact[:, :, :])
        nc.sync.dma_start(out=ACTS, in_=acts[:, :, :])
        halves = []
        for hx in range(2):
            cs = slice(hx * BH, (hx + 1) * BH)
            hh = {}
            hh["cs"] = cs
            hh["TA1"] = stp.tile([HA, BH], dt, name=f"TA1_{hx}")
            hh["TB1"] = stp.tile([KB, BH], dt, name=f"TB1_{hx}")
            hh["TA2"] = stp.tile([HA, BH], dt, name=f"TA2_{hx}")
            hh["TD"] = stp.tile([KD, BH], dt, name=f"TD_{hx}")
            hh["c1"] = stp.tile([HA, 2, BH], dt, name=f"c1_{hx}")
            hh["c2"] = stp.tile([HA, 2, BH], dt, name=f"c2_{hx}")
            hh["x1"] = stp.tile([KX, BH], dt, name=f"x1_{hx}")
            hh["TEF"] = stp.tile([HA, 2, BH], dt, name=f"TEF_{hx}")
            nc.sync.dma_start(out=hh["x1"][F:KX, :], in_=statzero[0:KX - F, cs])
            nc.sync.dma_start(out=hh["TD"][96 + A:KD, :],
                              in_=statzero[8:8 + A + 1, cs])
            nc.vector.memset(hh["TA1"], 0.0)
            nc.vector.memset(hh["TB1"], 0.0)
            nc.vector.memset(hh["TA2"], 0.0)
            nc.vector.memset(hh["TD"][0:96, :], 0.0)
            nc.vector.memset(hh["TEF"], 0.0)
            nc.vector.memset(hh["c1"], 0.0)
            nc.vector.memset(hh["c2"], 0.0)
            halves.append(hh)

        # zero all PSUM banks once: lanes the matmuls never write then
        # read as 0.0 forever (removes junk-lane hazards)
        for zb in range(8):
            pz = pp.tile([HA, 2, BH], f32, name=f"pz_{zb}", tag="g")
            nc.vector.memset(pz, 0.0)

        # gate column layout (permuted rows [i f o g], chunks a=128/b=72)
        GBASE = {"i": 0, "f": 200, "o": 400, "g": 600}

        def lstm_gates(kchunks, tag):
            """kchunks: list of (weight_key, rhs) accumulated in order.
            Per gate one [128, 2, BH] PSUM tile (one bank): slot0 = a-chunk
            (M=128), slot1 = b-chunk (M=72, lanes 72..127 junk)."""
            P = {}
            for gate in ("g", "i", "f", "o"):
                gp = pp.tile([HA, 2, BH], f32, name=f"P{gate}_{tag}", tag="g")
                for m, (mo, mn) in enumerate(((0, HA), (HA, H))):
                    ps = gp[0:mn - mo, m, :]
                    cols = slice(GBASE[gate] + mo, GBASE[gate] + mn)
                    last = len(kchunks) - 1
                    for j, (wk, rhs) in enumerate(kchunks):
                        nc.tensor.matmul(ps, W[wk][:, cols], rhs,
                                         start=(j == 0), stop=(j == last))
                P[gate] = gp
            return P

        def lstm_cell(P, c, ha, hb, tag):
            """update c (f32 [128,2,BH]) and h (ha [128,BH], hb [72,BH])"""
            gt = sp.tile([HA, 2, BH], dt, name=f"gt{tag}", tag="gt")
            sgi = sp.tile([HA, 2, BH], dt, name=f"sgi{tag}", tag="sgi")
            sgf = sp.tile([HA, 2, BH], dt, name=f"sgf{tag}", tag="sgf")
            sgo = sp.tile([HA, 2, BH], dt, name=f"sgo{tag}", tag="sgo")
            nc.scalar.activation(gt, P["g"], Tanh)
            nc.scalar.activation(sgi, P["i"], Sigmoid)
            nc.scalar.activation(sgf, P["f"], Sigmoid)
            nc.scalar.activation(sgo, P["o"], Sigmoid)
            ig = sp.tile([HA, 2, BH], dt, name=f"ig{tag}", tag="ig")
            fm = sp.tile([HA, 2, BH], dt, name=f"fm{tag}", tag="fm")
            nc.vector.tensor_mul(ig, sgi, gt)
            nc.vector.tensor_mul(fm, sgf, c)
            nc.vector.tensor_add(c, fm, ig)
            tch = sp.tile([HA, 2, BH], dt, name=f"tch{tag}", tag="tch")
            nc.scalar.activation(tch, c, Tanh)
            nc.vector.tensor_mul(ha, sgo[:, 0, :], tch[:, 0, :])
            nc.vector.tensor_mul(hb, sgo[0:HB, 1, :], tch[0:HB, 1, :])

        def emit_lstm1(hh, t, hx):
            x1_rhs = TACT[:, t, hh["cs"]] if t < CTX else hh["x1"]
            nc.vector.tensor_copy(hh["TD"][96:96 + A, :],
                                  ACTS[:, t, hh["cs"]])
            P1 = lstm_gates([("w1ha", hh["TA1"]), ("w1hb", hh["TB1"]),
                             ("w1x", x1_rhs)], f"1_{t}_{hx}")
            lstm_cell(P1, hh["c1"], hh["TA1"], hh["TB1"][0:HB, :],
                      f"1_{t}_{hx}")

        def emit_lstm2(hh, t, hx):
            P2 = lstm_gates([("w2h2a", hh["TA2"]), ("w2td", hh["TD"]),
                             ("w2h1a", hh["TA1"]), ("w2h1b", hh["TB1"])],
                            f"2_{t}_{hx}")
            lstm_cell(P2, hh["c2"], hh["TA2"], hh["TD"][0:HB, :],
                      f"2_{t}_{hx}")

        def emit_fc(hh, t, hx):
            x1_rhs = TACT[:, t, hh["cs"]] if t < CTX else hh["x1"]
            fcp = pp.tile([HA, 2, BH], f32, name=f"fcp_{t}_{hx}", tag="g")
            for m, (mo, mn) in enumerate(((0, HA), (HA, H))):
                ps = fcp[0:mn - mo, m, :]
                cols = slice(mo, mn)
                nc.tensor.matmul(ps, W["w3x"][:, cols], x1_rhs,
                                 start=True, stop=False)
                nc.tensor.matmul(ps, W["w3ha"][:, cols], hh["TA2"],
                                 start=False, stop=False)
                nc.tensor.matmul(ps, W["w3td"][:, cols], hh["TD"],
                                 start=False, stop=True)
            nc.scalar.activation(hh["TEF"], fcp, Tanh)
            f2p = pp.tile([F, BH], f32, name=f"f2p_{t}_{hx}", tag="g")
            nc.tensor.matmul(f2p, W["w4a"], hh["TEF"][:, 0, :],
                             start=True, stop=False)
            nc.tensor.matmul(f2p, W["w4b"], hh["TEF"][0:96, 1, :],
                             start=False, stop=True)
            stg = op.tile([F, BH], f32, name=f"stg_{t}_{hx}", tag="stg")
            if t < NSTEP - 1:
                nc.scalar.activation(hh["x1"][0:F, :], f2p, Tanh, bias=B4)
                nc.vector.tensor_copy(stg, hh["x1"][0:F, :])
            else:
                nc.scalar.activation(stg, f2p, Tanh, bias=B4)
            nc.gpsimd.dma_start(out=out[t - (CTX - 1), :, hh["cs"]], in_=stg)

        X, Y = halves
        # zipper the two independent half-batch recurrences: each half's fc
        # block is emitted inside the other half's LSTM1 window so the PE
        # FIFO always holds runnable matmuls during dependency chains
        for t in range(NSTEP):
            emit_lstm1(X, t, 0)
            if t - 1 >= CTX - 1:
                emit_fc(Y, t - 1, 1)
            emit_lstm1(Y, t, 1)
            emit_lstm2(X, t, 0)
            emit_lstm2(Y, t, 1)
            if t >= CTX - 1:
                emit_fc(X, t, 0)
        emit_fc(Y, NSTEP - 1, 1)

    nc.finalize()
    return nc


@functools.lru_cache(maxsize=1)
def _get_nc():
    return _build_nc()


def _prep_weights(W_ih1, W_hh1, b_ih1, b_hh1, W_ih2, W_hh2, b_ih2, b_hh2,
                  fc1_w, fc1_b, fc2_w, fc2_b):
    # gate rows reordered [i, f, o, g]
    perm = np.concatenate([np.arange(0, 200), np.arange(200, 400),
                           np.arange(600, 800), np.arange(400, 600)])
    W1p = np.asarray(W_ih1)[perm]          # [800, 48]
    W1hp = np.asarray(W_hh1)[perm]         # [800, 200]
    b1p = (np.asarray(b_ih1) + np.asarray(b_hh1))[perm]
    W2p = np.asarray(W_ih2)[perm]          # [800, 248]
    W2hp = np.asarray(W_hh2)[perm]         # [800, 200]
    b2p = (np.asarray(b_ih2) + np.asarray(b_hh2))[perm]
    Wt = W2p[:, 200:248]
    W2eff = Wt[:, 0:12] + Wt[:, 12:24] + Wt[:, 24:36] + Wt[:, 36:48]  # [800,12]
    fc1_w = np.asarray(fc1_w); fc1_b = np.asarray(fc1_b)
    fc2_w = np.asarray(fc2_w); fc2_b = np.asarray(fc2_b)
    npdt = _npdt()

    def c(x):
        return np.ascontiguousarray(x).astype(npdt)

    def pad_to(x, k):
        z = np.zeros((k, x.shape[1]), np.float32)
        z[:x.shape[0]] = x
        return z

    def td_weights(w_hb_T, w_as_T, bias):
        z = np.zeros((KD, w_hb_T.shape[1]), np.float32)
        z[0:HB] = w_hb_T
        z[96:108] = w_as_T
        z[108] = bias
        return z

    return {
        "w1ha": c(W1hp[:, 0:HA].T),
        "w1hb": c(pad_to(W1hp[:, HA:H].T, KB)),
        "w1x": c(pad_to(np.concatenate([W1p.T, b1p[None, :]], 0), KX)),
        "w2h2a": c(W2hp[:, 0:HA].T),
        "w2td": c(td_weights(W2hp[:, HA:H].T, W2eff.T, b2p)),
        "w2h1a": c(W2p[:, 0:HA].T),
        "w2h1b": c(pad_to(W2p[:, HA:H].T, KB)),
        "w3ha": c(fc1_w[:, 0:HA].T),
        "w3td": c(pad_to(fc1_w[:, HA:H].T, KD)),
        "w3x": c(pad_to(np.concatenate([fc1_w[:, 200:248].T,
                                        fc1_b[None, :]], 0), KX)),
        "w4a": c(fc2_w[:, 0:HA].T),
        "w4b": c(pad_to(fc2_w[:, HA:H].T, 96)),
        "b4": np.ascontiguousarray(fc2_b[:, None]).astype(np.float32),
    }


def kernel(tactiles, actions, W_ih1, W_hh1, b_ih1, b_hh1,
           W_ih2, W_hh2, b_ih2, b_hh2, fc1_w, fc1_b, fc2_w, fc2_b):
    global LAST_RESULT
    npdt = _npdt()
    tactiles = np.asarray(tactiles)
    actions = np.asarray(actions)

    wmap = _prep_weights(W_ih1, W_hh1, b_ih1, b_hh1, W_ih2, W_hh2, b_ih2, b_hh2,
                         fc1_w, fc1_b, fc2_w, fc2_b)

    in_maps = []
    for i in range(NCORES):
        s = slice(i * BL, (i + 1) * BL)
        tt = np.zeros((KX, CTX, BL), np.float32)
        tt[0:F] = tactiles[0:CTX, s, :].transpose(2, 0, 1)
        tt[F] = 1.0
        acts_T = np.ascontiguousarray(
            actions[1:T, s, :].transpose(2, 0, 1)).astype(npdt)   # [6,119,BL]
        sz = np.zeros((64, BL), np.float32)
        sz[0] = 1.0                      # x1 ones row
        sz[8:8 + A] = actions[0, s, :].T  # state rows
        sz[8 + A] = 1.0                  # TD ones row
        m = {"tact": tt.astype(npdt), "acts": acts_T,
             "statzero": sz.astype(npdt)}
        m.update(wmap)
        in_maps.append(m)

    nc = _get_nc()
    res = run_bass_kernel_spmd(nc, in_maps, core_ids=list(range(NCORES)))
    LAST_RESULT = res

    outs = [np.asarray(r["out"], dtype=np.float32) for r in res.results]
    full = np.concatenate([o.transpose(0, 2, 1) for o in outs], axis=1)
    return np.ascontiguousarray(full)
